# revision 34
# baseline (speedup 1.0000x reference)
"""Trainium2 Bass kernel for nn_GatedAtomUpdate (gnn_message_passing).

Strategy (no collectives needed):
  - Host sorts bonds by receiver atom and buckets them into 8 contiguous
    atom ranges (12500 atoms/core). Each core computes the gated MLP for
    its own bonds and segment-sums locally into its own atom slice; the
    host concatenates the 8 output slices. No all-reduce.
  - Bonds are packed into 128-bond tiles, each tile assigned to a single
    128-atom block (pad bonds carry rel_idx=255 so their one-hot row is
    all-zero and they contribute nothing, regardless of bias values).
  - Device pipeline per 2048-bond batch (m2), 16 tiles:
      L1:  2x row-packed pairs of K=64 matmuls: sub-batch features for
           the first 512 bonds of each 1024 live on partitions 0-63, the
           second 512 on 64-127; [W1|G1] stationary in row-groups
           (0,0)/(64,0) run concurrently, filling psum1[128(h|g), 1024].
      ACT: act1 = silu(psum1 + [b1;g1])  (FD=1024 calls, bf16 out)
      L2:  psum2[128 bonds, 16, 128] : per-tile MM with act1 tile as the
           stationary operand and blockdiag(W2,G2) as the moving operand
           -> bond-major [h2pre | g2pre]
      ACT: h2 = silu(h2pre), t = tanh(0.5*g2pre)   (FD=1024 calls)
      DVE: msg2 = (t + 1) * h2     == 2*silu(h2pre)*sigmoid(g2pre)
      SEG: GPSIMD broadcast-expands rel to [128,16,128] so the DVE
           one-hot compare (iota == rel) runs in 2x packed mode; PE
           matmul msg2^T @ S accumulates into a [64 feat, 128 atom] PSUM
           block; on block close DVE computes out = 0.5*pseg + atom.
  - All activation LUTs (silu, tanh) live in one table set -> one load.
"""

import math

import numpy as np
import ml_dtypes

import bass_rust
import concourse.bass as bass
import concourse.mybir as mybir
import concourse.tile as tile
from concourse.bass_utils import run_bass_kernel_spmd

# ---------------------------------------------------------------- constants
N_CORES = 8
D = 64
N_ATOMS = 100000
N_BONDS = 1500000
NA_CORE = N_ATOMS // N_CORES          # 12500
BLK = 128                             # atoms per block (one-hot width)
NBLK = math.ceil(NA_CORE / BLK)       # 98 blocks/core
NA_PAD = NBLK * BLK                   # 12544
TPB = 128                             # bonds per tile
L2B = 8                               # tiles per batch (1024 bonds)
XT_CHUNK_M2 = 16                      # m2 batches per xt DMA chunk
S8_CHUNK_M2 = 8                       # m2 batches per one-hot DMA chunk

_BF16 = mybir.dt.bfloat16
_F32 = mybir.dt.float32


# ------------------------------------------------------- walrus workaround
def _patched_drain_and_barrier(self, tick_clock, wait_clock):
    """This walrus build accepts at most ONE sync-wait on TPB_CTRL-class
    instructions (Drain/NoOp), but TileContext's exit drain attaches one
    wait per DMA completion lane. Emit the waits on single-wait NOPs on
    the same engine first (program order gives the same guarantee), leave
    the drain bare, and reset semaphores one at a time."""
    nc = self.nc
    gc = tick_clock.global_clock
    ticks = list(gc)
    n = len(ticks)
    for proc, t in enumerate(ticks):
        if t > 0:
            vcp = bass_rust.VectorClock([t if j == proc else 0 for j in range(n)])
            nop = nc.sync.nop()
            wait_clock.add_sem_waits(nop.ins, tile.ScopedClock({None: vcp}))
    nc.sync.drain()
    nc.all_engine_barrier()
    assert self.sems is not None
    popped = nc._tile_sem_poison_stack.pop()
    assert popped is self._sem_poison
    for s in list(self.sems.allocated().values()):
        nc.clear_and_free_semaphores([s])
    nc.all_engine_barrier()


tile.TileContext._drain_and_barrier = _patched_drain_and_barrier


def _split_multi_waits(bir):
    """This walrus build rejects >1 sync-wait on an instruction ('Too many
    sync wait commands'). Move extra waits onto fresh single-wait NoOps
    inserted immediately before the instruction on the same engine —
    program order on the engine's sequencer preserves semantics."""
    n_new = 0
    for fn in bir.get("functions", []):
        for bb in fn.get("blocks", []):
            insts = bb.get("instructions", [])
            out = []
            for inst in insts:
                si = inst.get("sync_info") or {}
                ow = si.get("on_wait") or []
                if len(ow) > 1:
                    for i, w in enumerate(ow[:-1]):
                        out.append({
                            "name": f"{inst['name']}_sw{i}",
                            "opcode": "NoOp",
                            "engine": inst["engine"],
                            "ins": [],
                            "outs": [],
                            "sync_info": {"on_update": [], "on_wait": [w]},
                            "debug": inst.get("debug", 0),
                        })
                        n_new += 1
                    si["on_wait"] = [ow[-1]]
                out.append(inst)
            bb["instructions"] = out
    return n_new


_orig_to_json_bytes = bass.Bass.to_json_bytes


def _to_json_bytes_patched(self, *args, **kwargs):
    import json as _json
    raw = _orig_to_json_bytes(self, *args, **kwargs)
    bir = _json.loads(raw)
    n = _split_multi_waits(bir)
    if n == 0:
        return raw
    return _json.dumps(bir).encode()


bass.Bass.to_json_bytes = _to_json_bytes_patched


# ------------------------------------------------------------ host sharding
GBLK = math.ceil(N_ATOMS / BLK)       # 782 global atom blocks (last partial)


def _plan_and_pack(atom_features, bond_features, bond_atom_indices):
    """Sort bonds by receiver and bucket into 128-atom global blocks. Deal
    blocks to (core, slot) pairs in sorted-count groups of 8 so the shared
    SPMD tile schedule (max over cores per slot) wastes minimal padding.
    Returns the schedule, per-core packed arrays, and the block assignment
    matrix needed to unshard the output."""
    recv = bond_atom_indices[:, 1].astype(np.int64)
    order = np.argsort(recv, kind="stable")
    sorted_recv = recv[order]
    gb_of = sorted_recv // BLK
    r_of = (sorted_recv % BLK).astype(np.float32)
    gcnt = np.bincount(gb_of, minlength=GBLK)
    gstart = np.concatenate([[0], np.cumsum(gcnt)[:-1]]).astype(np.int64)

    # deal blocks: sort descending by count, group consecutive 8 into one
    # slot (one block per core); pad with virtual empty blocks
    srt = np.argsort(-gcnt, kind="stable")
    nvirt = N_CORES * NBLK - GBLK
    dealt = np.concatenate([srt, np.full(nvirt, -1, dtype=np.int64)])
    B = dealt.reshape(NBLK, N_CORES).T          # B[c, k] = global block id

    cnt = np.where(B >= 0, gcnt[np.clip(B, 0, None)], 0)   # [N_CORES, NBLK]

    # shared tile schedule: tiles per slot (>=1 so every block is written)
    T = np.maximum(1, -(-cnt.max(axis=0) // TPB))
    ntiles = int(T.sum())
    pad_tiles = (-ntiles) % L2B
    T[-1] += pad_tiles
    ntiles += pad_tiles
    tstart = np.concatenate([[0], np.cumsum(T)[:-1]]).astype(np.int64)

    # slot id for every tile, in order
    tile_block = np.repeat(np.arange(NBLK), T)

    xt_list, s8_list, atomT_list = [], [], []
    nslots = ntiles * TPB
    ngrp = nslots // 1024
    oh_iota = np.arange(BLK, dtype=np.float32)
    for c in range(N_CORES):
        slot_l, gather_l, rel_l = [], [], []
        ap = np.zeros((NA_PAD, D), dtype=np.float32)
        for k in range(NBLK):
            gb = B[c, k]
            if gb < 0:
                continue
            n = int(gcnt[gb])
            lo = gstart[gb]
            if n:
                slot_l.append(tstart[k] * TPB + np.arange(n))
                gather_l.append(order[lo:lo + n])
                rel_l.append(r_of[lo:lo + n])
            a0 = gb * BLK
            a1 = min(a0 + BLK, N_ATOMS)
            ap[k * BLK:k * BLK + (a1 - a0)] = atom_features[a0:a1]

        slot = np.concatenate(slot_l)
        gather = np.concatenate(gather_l)
        relv = np.concatenate(rel_l)

        rel = np.full(nslots, 255, dtype=np.float32)
        rel[slot] = relv
        x_slot = np.zeros((nslots, D), dtype=np.float32)
        x_slot[slot] = bond_features[gather]

        # row-packed layout: [128, nslots/2] where partitions 0-63 carry the
        # features of the first 512 bonds of each 1024-group and partitions
        # 64-127 carry the second 512.
        xs = x_slot.reshape(ngrp, 2, 512, D)
        xt = np.ascontiguousarray(
            xs.transpose(1, 3, 0, 2).reshape(2 * D, ngrp * 512)
        ).astype(ml_dtypes.bfloat16)
        # one-hot S8 [TPB, ntiles, BLK]: host-built, streamed to the device
        # as the seg-matmul moving operand (pad slots rel=255 -> all-zero)
        relT = rel.reshape(ntiles, TPB).T
        s8 = (relT[:, :, None] == oh_iota).astype(ml_dtypes.bfloat16)
        xt_list.append(xt)
        s8_list.append(np.ascontiguousarray(s8.reshape(TPB, ntiles * BLK)))
        atomT_list.append(np.ascontiguousarray(ap.T))

    return ntiles, tile_block, xt_list, s8_list, atomT_list, B


def _pack_weights(W1, G1, W2, G2, b1, g1, b2, g2):
    wg1_row = np.concatenate([W1, G1], axis=1)              # [64, 128]
    wg1 = np.concatenate([wg1_row, wg1_row], axis=0)        # [128, 128]
    wg2 = np.zeros((2 * D, 2 * D), dtype=np.float32)
    wg2[:D, :D] = W2
    wg2[D:, D:] = G2
    b1g1 = np.concatenate([b1, g1]).reshape(2 * D, 1).astype(np.float32)
    b2g2 = np.concatenate([b2, g2]).reshape(1, 2 * D)
    return (
        wg1.astype(ml_dtypes.bfloat16),
        wg2.astype(ml_dtypes.bfloat16),
        b1g1,
        b2g2.astype(ml_dtypes.bfloat16),
    )


# ------------------------------------------------------------- device kernel
def _build_nc(ntiles, tile_block, has_bias2):
    nb2 = ntiles // L2B
    nc = bass.Bass()

    xt_d = nc.dram_tensor("xt", [2 * D, ntiles * TPB // 2], _BF16, kind="ExternalInput")
    s8_d = nc.dram_tensor("s8", [TPB, ntiles * BLK], _BF16, kind="ExternalInput")
    atomT_d = nc.dram_tensor("atomT", [D, NA_PAD], _F32, kind="ExternalInput")
    wg1_d = nc.dram_tensor("wg1", [2 * D, 2 * D], _BF16, kind="ExternalInput")
    wg2_d = nc.dram_tensor("wg2", [2 * D, 2 * D], _BF16, kind="ExternalInput")
    b1g1_d = nc.dram_tensor("b1g1", [2 * D, 1], _F32, kind="ExternalInput")
    b2g2_d = nc.dram_tensor("b2g2", [1, 2 * D], _BF16, kind="ExternalInput")
    out_d = nc.dram_tensor("out", [D, NA_PAD], _F32, kind="ExternalOutput")

    AF = mybir.ActivationFunctionType

    # first/last tile flags per block
    first_of_block = np.zeros(ntiles, dtype=bool)
    last_of_block = np.zeros(ntiles, dtype=bool)
    prev = -1
    for t in range(ntiles):
        b = tile_block[t]
        if b != prev:
            first_of_block[t] = True
            if t > 0:
                last_of_block[t - 1] = True
            prev = b
    last_of_block[ntiles - 1] = True

    with tile.TileContext(nc) as tc:
        with (
            tc.tile_pool(name="singles", bufs=1) as singles,
            tc.tile_pool(name="xtp", bufs=3) as xtp,
            tc.tile_pool(name="s8p", bufs=3) as s8p,
            tc.tile_pool(name="actp", bufs=2) as actp,
            tc.tile_pool(name="l2p", bufs=3) as l2p,
            tc.tile_pool(name="outp", bufs=4) as outp,
            tc.tile_pool(name="psum1p", bufs=1, space="PSUM") as psum1p,
            tc.tile_pool(name="psum2p", bufs=2, space="PSUM") as psum2p,
            tc.tile_pool(name="psegp", bufs=2, space="PSUM") as psegp,
        ):
            wg1_sb = singles.tile([2 * D, 2 * D], _BF16)
            nc.sync.dma_start(out=wg1_sb[:], in_=wg1_d[:, :])
            wg2_sb = singles.tile([2 * D, 2 * D], _BF16)
            nc.sync.dma_start(out=wg2_sb[:], in_=wg2_d[:, :])
            b1g1_sb = singles.tile([2 * D, 1], _F32)
            nc.sync.dma_start(out=b1g1_sb[:], in_=b1g1_d[:, :])
            b2g2_sb = singles.tile([1, 2 * D], _BF16)
            nc.sync.dma_start(out=b2g2_sb[:], in_=b2g2_d[:, :])
            ones_sb = singles.tile([1, 2 * D], _BF16)
            nc.vector.memset(ones_sb[:], 1.0)
            atom_sb = singles.tile([D, NA_PAD], _F32)

            pseg_cur = None
            stash = {}

            def emit_seg(j):
                nonlocal pseg_cur
                msg_j, S8_j = stash.pop(j)
                t0j = j * L2B
                for tt in range(L2B):
                    t_glob = t0j + tt
                    b = int(tile_block[t_glob])
                    if first_of_block[t_glob]:
                        pseg_cur = psegp.tile([D, BLK], _F32, tag="pseg")
                    nc.tensor.matmul(
                        pseg_cur[:, :], msg_j[:, tt, :], S8_j[:, tt, :],
                        start=bool(first_of_block[t_glob]),
                        stop=bool(last_of_block[t_glob]),
                    )
                    if last_of_block[t_glob]:
                        # out = 0.25 * pseg + atom  (msg4 carries 4x message)
                        ob = outp.tile([D, BLK], _F32, tag="ob")
                        nc.vector.scalar_tensor_tensor(
                            ob[:],
                            pseg_cur[:, :],
                            0.25,
                            atom_sb[:, b * BLK:(b + 1) * BLK],
                            mybir.AluOpType.mult,
                            mybir.AluOpType.add,
                        )
                        # stream the finished block out now; avoids one big
                        # serial DMA after the last block closes
                        nc.sync.dma_start(
                            out=out_d[:, b * BLK:(b + 1) * BLK],
                            in_=ob[:],
                        )

            psum2s = {}

            def emit_l2act(j):
                """L2 activations for batch j (run one iter after its L2
                fills psum2, so ACT never waits on the act1->L2->h2 chain).

                One FD=1024 tanh covers BOTH halves: T = tanh(0.5*psum2).
                Then  silu(h)   = 0.5*h*(1+T_h)
                      sigmoid(g) = 0.5*(1+T_g)
                so    msg4 := (1+T_h)*h2pre * (1+T_g) = 4*silu(h)*sigmoid(g)
                and the block-close scales by 0.25."""
                psum2 = psum2s.pop(j)
                tg = l2p.tile([TPB, L2B, 2 * D], _BF16, tag="tg")
                nc.scalar.activation(tg[:], psum2[:], AF.Tanh, scale=0.5)
                u = l2p.tile([TPB, L2B, D], _BF16, tag="u")
                nc.vector.scalar_tensor_tensor(
                    u[:], tg[:, :, 0:D], 1.0, psum2[:, :, 0:D],
                    mybir.AluOpType.add, mybir.AluOpType.mult,
                )
                msg = l2p.tile([TPB, L2B, D], _BF16, tag="msg")
                nc.vector.scalar_tensor_tensor(
                    msg[:], tg[:, :, D:2 * D], 1.0, u[:],
                    mybir.AluOpType.add, mybir.AluOpType.mult,
                )
                stash[j] = (msg, stash[j])

            # chunk c covers m2 range [starts[c], starts[c+1]); the first
            # chunk is a single batch so compute starts almost immediately
            starts = [0, 1] + list(range(XT_CHUNK_M2, nb2, XT_CHUNK_M2)) + [nb2]
            m2_chunk_start = {}
            for c in range(len(starts) - 1):
                for m in range(starts[c], starts[c + 1]):
                    m2_chunk_start[m] = starts[c]

            xt_sbs = {}

            def emit_chunk(s, e):
                xt_sb = xtp.tile([2 * D, XT_CHUNK_M2 * 512], _BF16, tag="xt")
                nc.sync.dma_start(
                    out=xt_sb[:, :(e - s) * 512],
                    in_=xt_d[:, s * 512: e * 512],
                )
                xt_sbs[s] = xt_sb

            s8_sbs = {}

            def emit_s8_chunk(s):
                e = min(s + S8_CHUNK_M2, nb2)
                s8_sb = s8p.tile(
                    [TPB, S8_CHUNK_M2, L2B, BLK], _BF16, tag="s8"
                )
                nc.sync.dma_start(
                    out=s8_sb[:, :e - s, :, :],
                    in_=s8_d[:, s * L2B * BLK: e * L2B * BLK],
                )
                s8_sbs[s] = s8_sb

            acts = {}

            def emit_l1(j):
                s = m2_chunk_start[j]
                if s not in xt_sbs:
                    ci = starts.index(s)
                    emit_chunk(s, starts[ci + 1])
                xt_sb = xt_sbs[s]
                co = (j - s) * 512
                psum1 = psum1p.tile([2 * D, 1024], _F32, tag="psum1")
                nc.tensor.matmul(
                    psum1[:, 0:512], wg1_sb[0:D, :], xt_sb[0:D, co:co + 512],
                    start=True, stop=True,
                )
                nc.tensor.matmul(
                    psum1[:, 512:1024], wg1_sb[D:2 * D, :],
                    xt_sb[D:2 * D, co:co + 512],
                    start=True, stop=True,
                )
                act1 = actp.tile([2 * D, 1024], _BF16, tag="act1")
                nc.scalar.activation(
                    act1[:], psum1[:], AF.Silu, bias=b1g1_sb[:, 0:1], scale=1.0
                )
                acts[j] = act1

            # xt chunks for batches 0..15 land before the big atom DMA
            emit_chunk(0, 1)
            emit_chunk(1, XT_CHUNK_M2)
            emit_s8_chunk(0)
            nc.sync.dma_start(out=atom_sb[:], in_=atomT_d[:, :])
            emit_s8_chunk(S8_CHUNK_M2)
            emit_s8_chunk(2 * S8_CHUNK_M2)
            emit_l1(0)
            chunk_starts = set(starts[:-1])
            for m2 in range(nb2):
                # prefetch the S8 chunk ~14 iters ahead of its first use
                nxt = m2 + 2 * S8_CHUNK_M2 - 2
                if nxt % S8_CHUNK_M2 == 0 and nxt < nb2 and nxt not in s8_sbs:
                    emit_s8_chunk(nxt)
                # prefetch the xt chunk ~14 iters ahead of its first use
                xnxt = m2 + 14
                if xnxt in chunk_starts and xnxt not in xt_sbs:
                    ci = starts.index(xnxt)
                    emit_chunk(xnxt, starts[ci + 1])
                # ---- L1 + act1 for the NEXT batch first: the PE runs it
                # before this batch's L2, so act1(m2+1) is ready the moment
                # the scalar engine finishes h2/tg of batch m2-1
                if m2 + 1 < nb2:
                    emit_l1(m2 + 1)

                # ---- L2: per-tile stationary=act1 tile, moving=blockdiag W2|G2
                act1 = acts.pop(m2)
                psum2 = psum2p.tile([TPB, L2B, 2 * D], _F32, tag="psum2")
                for tt in range(L2B):
                    sl = tt * TPB
                    nc.tensor.matmul(
                        psum2[:, tt, :], act1[:, sl:sl + TPB], wg2_sb[:, :],
                        start=True, stop=not has_bias2,
                    )
                    if has_bias2:
                        nc.tensor.matmul(
                            psum2[:, tt, :], ones_sb[0:1, :], b2g2_sb[0:1, :],
                            start=False, stop=True,
                        )
                psum2s[m2] = psum2

                # ---- one-hot slab for this batch (host-built, DMA-streamed)
                s8c = s8_sbs[(m2 // S8_CHUNK_M2) * S8_CHUNK_M2]
                stash[m2] = s8c[:, m2 % S8_CHUNK_M2, :, :]

                # ---- skewed stages: L2-activations for batch m2-1, segment
                # accumulation for batch m2-2
                if m2 > 0:
                    emit_l2act(m2 - 1)
                if m2 > 1:
                    emit_seg(m2 - 2)
            emit_l2act(nb2 - 1)
            emit_seg(nb2 - 2)
            emit_seg(nb2 - 1)

    return nc


# ----------------------------------------------------------------- kernel()
LAST_EXEC_NS = None
LAST_SIM_NS = None
LAST_RESULT = None


def kernel(**inputs):
    atom_features = np.asarray(inputs["atom_features"], dtype=np.float32)
    bond_features = np.asarray(inputs["bond_features"], dtype=np.float32)
    bond_atom_indices = np.asarray(inputs["bond_atom_indices"])
    W1 = np.asarray(inputs["W1"], dtype=np.float32)
    W2 = np.asarray(inputs["W2"], dtype=np.float32)
    G1 = np.asarray(inputs["G1"], dtype=np.float32)
    G2 = np.asarray(inputs["G2"], dtype=np.float32)
    b1 = np.asarray(inputs["b1"], dtype=np.float32)
    b2 = np.asarray(inputs["b2"], dtype=np.float32)
    g1 = np.asarray(inputs["g1"], dtype=np.float32)
    g2 = np.asarray(inputs["g2"], dtype=np.float32)

    ntiles, tile_block, xt_list, s8_list, atomT_list, B = _plan_and_pack(
        atom_features, bond_features, bond_atom_indices
    )
    wg1, wg2, b1g1, b2g2 = _pack_weights(W1, G1, W2, G2, b1, g1, b2, g2)
    has_bias2 = not (np.all(b2 == 0.0) and np.all(g2 == 0.0))

    nc = _build_nc(ntiles, tile_block, has_bias2)

    in_maps = []
    for c in range(N_CORES):
        in_maps.append({
            "xt": xt_list[c],
            "s8": s8_list[c],
            "atomT": atomT_list[c],
            "wg1": wg1,
            "wg2": wg2,
            "b1g1": b1g1,
            "b2g2": b2g2,
        })

    import os as _os
    global LAST_EXEC_NS, LAST_SIM_NS, LAST_RESULT
    try:
        from concourse.timeline_sim import TimelineSim
        LAST_SIM_NS = TimelineSim(nc, trace=False).simulate()
    except Exception:
        LAST_SIM_NS = None

    _trace = bool(int(_os.environ.get("KERNEL_TRACE", "0")))
    res = run_bass_kernel_spmd(nc, in_maps, core_ids=list(range(N_CORES)), trace=_trace)
    LAST_EXEC_NS = res.exec_time_ns
    LAST_RESULT = res

    out = np.empty((N_ATOMS, D), dtype=np.float32)
    for c in range(N_CORES):
        oc = res.results[c]["out"]
        for k in range(NBLK):
            gb = B[c, k]
            if gb < 0:
                continue
            a0 = gb * BLK
            n = min(BLK, N_ATOMS - a0)
            out[a0:a0 + n] = oc[:, k * BLK:k * BLK + n].T
    return out
